# revision 3
# baseline (speedup 1.0000x reference)
"""4-bit comparator (SNN logic-gate network) as a Trainium2 Bass kernel.

Full inputs A, B: [4194304, 4] float32 binary (0/1), column 0 = MSB.
Outputs: (a_gt_b, a_eq_b) each [N, 1] float32 0/1, where
  a_gt_b = 1.0 iff int4(A) > int4(B),  a_eq_b = 1.0 iff all 4 bits equal.

Device-side math (mode "m"):
  Host packs each row's 4 bits losslessly: [c0,c1,c2,c3] -> bytes
  [c3,c2,c1,c0] (column flip folded into the f32->u8 cast), viewed as two
  little-endian uint16: lo = c3 + 256*c2, hi = c1 + 256*c0. Per core the
  lo/hi halves are laid out as contiguous per-tile blocks so one DMA per
  input tile delivers both halves as packed tile halves.

  On the DVE (all values < 2^24, exact in its f32-internal ALU):
    p  = 512*hi(A) + lo(A)      (scalar_tensor_tensor)
    q  = 512*hi(B) + lo(B)      (scalar_tensor_tensor)
    gt = (p > q) ; eq = (p == q)  (tensor_tensor compares, int8 out)
  p/q are monotone (lexicographic) encodings of the 4-bit values.

HBM traffic per core: 2 MiB A + 2 MiB B (u16) + 1 MiB output (i8)
= 5 MiB, 2.5x less than the f32 bit layout. gt/eq are written as one
[gt block | eq block] tensor -> 3 DMAs per tile total.

Sharding: data-parallel over rows across 8 NeuronCores (524288 rows/core).
"""

import contextlib
import functools
import sys

sys.path.insert(0, "/opt/trn_rl_repo")

import numpy as np

import concourse.tile as tile
from concourse import bacc, mybir
from concourse.alu_op_type import AluOpType
from concourse.bass_utils import run_bass_kernel_spmd

P = 128
N_CORES = 8
R = 1024          # rows per partition per tile -> 4 tiles per core
BUFS_IO = 4
BUFS_TMP = 3
BUFS_OUT = 4


def build_nc(S: int, reps: int = 1, internal_out: bool = False,
             unroll: bool = True):
    """Single-core program. reps>1 repeats the pipeline in-NEFF (unrolled,
    benchmarking only); internal_out=True keeps GT/EQ writes in DRAM but
    exposes only a 1-byte ExternalOutput so benchmark calls fetch nothing."""
    rows_per_tile = P * R
    assert S % rows_per_tile == 0, (S, rows_per_tile)
    n_tiles = S // rows_per_tile
    u16 = mybir.dt.uint16
    f32 = mybir.dt.float32
    i8 = mybir.dt.int8

    nc = bacc.Bacc("TRN2", target_bir_lowering=False, debug=False)
    out_kind = "Internal" if internal_out else "ExternalOutput"
    # Per tile, [lo block | hi block], each P*R u16.
    A = nc.dram_tensor("A", [2 * S, 1], u16, kind="ExternalInput").ap()
    B = nc.dram_tensor("B", [2 * S, 1], u16, kind="ExternalInput").ap()
    Av = A.rearrange("(n two p r) j -> n p two (r j)", two=2, p=P, r=R)
    Bv = B.rearrange("(n two p r) j -> n p two (r j)", two=2, p=P, r=R)
    # Per tile, [gt block | eq block], each P*R i8.
    GE = nc.dram_tensor("GE", [2 * S, 1], i8, kind=out_kind).ap()
    GEv = GE.rearrange("(n two p r) j -> n p two (r j)", two=2, p=P, r=R)

    with tile.TileContext(nc) as tc:
        with (
            tc.tile_pool(name="io", bufs=BUFS_IO) as io,
            tc.tile_pool(name="tmp", bufs=BUFS_TMP) as tmp,
            tc.tile_pool(name="outp", bufs=BUFS_OUT) as outp,
        ):
            # Loads on the sync HWDGE ring, stores on the scalar ring.
            if reps > 1 and not unroll:
                loop_cm = tc.For_i(0, reps, 1)
                outer = 1
            else:
                loop_cm = contextlib.nullcontext()
                outer = reps
            with loop_cm:
                for _ in range(outer):
                    for t in range(n_tiles):
                        ta = io.tile([P, 2 * R], u16, tag="ta")
                        nc.sync.dma_start(
                            ta[:].rearrange("p (two r) -> p two r", two=2),
                            Av[t])
                        tb = io.tile([P, 2 * R], u16, tag="tb")
                        nc.sync.dma_start(
                            tb[:].rearrange("p (two r) -> p two r", two=2),
                            Bv[t])
                        pt = tmp.tile([P, R], f32, tag="p")
                        nc.vector.scalar_tensor_tensor(
                            pt[:], ta[:, R:], 512.0, ta[:, :R],
                            AluOpType.mult, AluOpType.add,
                        )
                        qt = tmp.tile([P, R], f32, tag="q")
                        nc.vector.scalar_tensor_tensor(
                            qt[:], tb[:, R:], 512.0, tb[:, :R],
                            AluOpType.mult, AluOpType.add,
                        )
                        ge_t = outp.tile([P, 2 * R], i8, tag="ge")
                        nc.vector.tensor_tensor(ge_t[:, :R], pt[:], qt[:],
                                                AluOpType.is_gt)
                        nc.vector.tensor_tensor(ge_t[:, R:], pt[:], qt[:],
                                                AluOpType.is_equal)
                        nc.scalar.dma_start(
                            GEv[t],
                            ge_t[:].rearrange("p (two r) -> p two r", two=2))
        if internal_out:
            OUT = nc.dram_tensor("OUT", [1, 1], i8, kind="ExternalOutput").ap()
            nc.sync.dma_start(OUT[:], ge_t[0:1, 0:1])
    nc.compile()
    return nc


def _to_u16(X: np.ndarray, N_pad: int) -> np.ndarray:
    """f32 [N,4] (col 0 = MSB) -> uint16 [N_pad,2] (lo, hi) encoding."""
    Xb = X[:, ::-1].astype(np.uint8)          # one pass: flip + cast
    V = Xb.view(np.uint16)                    # [N,2]: lo=c3+256c2, hi=c1+256c0
    if N_pad != X.shape[0]:
        V = np.pad(V, ((0, N_pad - X.shape[0]), (0, 0)))
    return V


def prep_in_maps(A: np.ndarray, B: np.ndarray):
    """Pad, pack, shard. -> (in_maps, S)"""
    A = np.asarray(A, dtype=np.float32)
    B = np.asarray(B, dtype=np.float32)
    N = A.shape[0]
    chunk = N_CORES * P * R
    N_pad = -(-N // chunk) * chunk
    S = N_pad // N_CORES
    n_tiles = S // (P * R)
    VA = _to_u16(A, N_pad)
    VB = _to_u16(B, N_pad)

    def _m(V, i):
        # [S,2] u16 -> per-tile [lo block | hi block] layout, each P*R
        X = V[i * S : (i + 1) * S].reshape(n_tiles, P, R, 2)
        return np.ascontiguousarray(X.transpose(0, 3, 1, 2)).reshape(2 * S, 1)

    in_maps = [{"A": _m(VA, i), "B": _m(VB, i)} for i in range(N_CORES)]
    return in_maps, S


@functools.lru_cache(maxsize=None)
def _get_nc(S: int):
    return build_nc(S)


@functools.lru_cache(maxsize=None)
def bench_nc(S: int, reps: int):
    return build_nc(S, reps=reps, internal_out=True)


def kernel(A: np.ndarray, B: np.ndarray):
    N = np.asarray(A).shape[0]
    in_maps, S = prep_in_maps(A, B)
    nc = _get_nc(S)
    res = run_bass_kernel_spmd(nc, in_maps, list(range(N_CORES)))
    n_tiles = S // (P * R)
    gts, eqs = [], []
    for r in res.results:
        GE = r["GE"].reshape(n_tiles, 2, P * R)
        gts.append(GE[:, 0].reshape(S, 1))
        eqs.append(GE[:, 1].reshape(S, 1))
    gt = np.concatenate(gts, axis=0)[:N]
    eq = np.concatenate(eqs, axis=0)[:N]
    return gt.astype(np.float32), eq.astype(np.float32)


# revision 7
# speedup vs baseline: 1.4731x; 1.4731x over previous
"""4-bit comparator (SNN logic-gate network) as a Trainium2 Bass kernel.

Full inputs A, B: [4194304, 4] float32 binary (0/1), column 0 = MSB.
Outputs: (a_gt_b, a_eq_b) each [N, 1] float32 0/1, where
  a_gt_b = 1.0 iff int4(A) > int4(B),  a_eq_b = 1.0 iff all 4 bits equal.

Device-side math (mode "m"):
  Host packs each row's 4 bits losslessly: [c0,c1,c2,c3] -> bytes
  [c3,c2,c1,c0] (column flip folded into the f32->u8 cast), viewed as two
  little-endian uint16: lo = c3 + 256*c2, hi = c1 + 256*c0. Per core the
  lo/hi halves are laid out as contiguous per-tile blocks so one DMA per
  input tile delivers both halves as packed tile halves.

  Compute (all values < 2^24, exact in the f32-internal ALUs), split
  across DVE and ACT so the two engines overlap:
    DVE: d  = A - B per half (u16 - u16 -> f16, packed 2x mode)
         df = 512*d_hi + d_lo        (monotone: sign(df) = cmp result)
         gt = (df > 0)               (int8 out)
    ACT: ab = |df| ; eq = Relu(1 - ab)   (exact: |df| is 0 or >= 1)

HBM traffic per core: 2 MiB A + 2 MiB B (u16) + 1 MiB output (i8)
= 5 MiB, 2.5x less than the f32 bit layout. gt/eq are written as one
[gt block | eq block] tensor -> 3 DMAs per tile total.

Sharding: data-parallel over rows across 8 NeuronCores (524288 rows/core).
"""

import contextlib
import functools
import sys

sys.path.insert(0, "/opt/trn_rl_repo")

import numpy as np

import bass_rust
import concourse.tile as tile
from concourse import bacc, mybir
from concourse.alu_op_type import AluOpType
from concourse.bass_utils import run_bass_kernel_spmd

ACT_F = bass_rust.ActivationFunctionType

P = 128
N_CORES = 8
R = 1024          # rows per partition per tile -> 4 tiles per core
BUFS_IO = 4
BUFS_TMP = 3
BUFS_OUT = 4


def build_nc(S: int, reps: int = 1, internal_out: bool = False,
             unroll: bool = True):
    """Single-core program. reps>1 repeats the pipeline in-NEFF (unrolled,
    benchmarking only); internal_out=True keeps GT/EQ writes in DRAM but
    exposes only a 1-byte ExternalOutput so benchmark calls fetch nothing."""
    rows_per_tile = P * R
    assert S % rows_per_tile == 0, (S, rows_per_tile)
    n_tiles = S // rows_per_tile
    u16 = mybir.dt.uint16
    f16 = mybir.dt.float16
    f32 = mybir.dt.float32
    i8 = mybir.dt.int8

    nc = bacc.Bacc("TRN2", target_bir_lowering=False, debug=False)
    out_kind = "Internal" if internal_out else "ExternalOutput"
    # Per tile, [lo block | hi block], each P*R u16.
    A = nc.dram_tensor("A", [2 * S, 1], u16, kind="ExternalInput").ap()
    B = nc.dram_tensor("B", [2 * S, 1], u16, kind="ExternalInput").ap()
    Av = A.rearrange("(n two p r) j -> n p two (r j)", two=2, p=P, r=R)
    Bv = B.rearrange("(n two p r) j -> n p two (r j)", two=2, p=P, r=R)
    # Per tile, [gt block | eq block], each P*R i8.
    GE = nc.dram_tensor("GE", [2 * S, 1], i8, kind=out_kind).ap()
    GEv = GE.rearrange("(n two p r) j -> n p two (r j)", two=2, p=P, r=R)

    with tile.TileContext(nc) as tc:
        with (
            tc.tile_pool(name="io", bufs=BUFS_IO) as io,
            tc.tile_pool(name="tmp", bufs=BUFS_TMP) as tmp,
            tc.tile_pool(name="outp", bufs=BUFS_OUT) as outp,
        ):
            # Loads on the sync HWDGE ring, stores on the scalar ring.
            if reps > 1 and not unroll:
                loop_cm = tc.For_i(0, reps, 1)
                outer = 1
            else:
                loop_cm = contextlib.nullcontext()
                outer = reps
            with loop_cm:
                for _ in range(outer):
                    for t in range(n_tiles):
                        ta = io.tile([P, 2 * R], u16, tag="ta")
                        nc.sync.dma_start(
                            ta[:].rearrange("p (two r) -> p two r", two=2),
                            Av[t])
                        tb = io.tile([P, 2 * R], u16, tag="tb")
                        nc.sync.dma_start(
                            tb[:].rearrange("p (two r) -> p two r", two=2),
                            Bv[t])
                        d = tmp.tile([P, 2 * R], f16, tag="d")
                        nc.vector.tensor_tensor(d[:], ta[:], tb[:],
                                                AluOpType.subtract)
                        df = tmp.tile([P, R], f32, tag="df")
                        nc.vector.scalar_tensor_tensor(
                            df[:], d[:, R:], 512.0, d[:, :R],
                            AluOpType.mult, AluOpType.add,
                        )
                        ge_t = outp.tile([P, 2 * R], i8, tag="ge")
                        nc.vector.tensor_scalar(ge_t[:, :R], df[:], 0.0, None,
                                                AluOpType.is_gt)
                        ab = tmp.tile([P, R], f16, tag="ab")
                        nc.scalar.activation(ab[:], df[:], ACT_F.Abs)
                        nc.scalar.activation(ge_t[:, R:], ab[:], ACT_F.Relu,
                                             bias=1.0, scale=-1.0)
                        nc.scalar.dma_start(
                            GEv[t],
                            ge_t[:].rearrange("p (two r) -> p two r", two=2))
        if internal_out:
            OUT = nc.dram_tensor("OUT", [1, 1], i8, kind="ExternalOutput").ap()
            nc.sync.dma_start(OUT[:], ge_t[0:1, 0:1])
    nc.compile()
    return nc


def _to_u16(X: np.ndarray, N_pad: int) -> np.ndarray:
    """f32 [N,4] (col 0 = MSB) -> uint16 [N_pad,2] (lo, hi) encoding."""
    Xb = X[:, ::-1].astype(np.uint8)          # one pass: flip + cast
    V = Xb.view(np.uint16)                    # [N,2]: lo=c3+256c2, hi=c1+256c0
    if N_pad != X.shape[0]:
        V = np.pad(V, ((0, N_pad - X.shape[0]), (0, 0)))
    return V


def prep_in_maps(A: np.ndarray, B: np.ndarray):
    """Pad, pack, shard. -> (in_maps, S)"""
    A = np.asarray(A, dtype=np.float32)
    B = np.asarray(B, dtype=np.float32)
    N = A.shape[0]
    chunk = N_CORES * P * R
    N_pad = -(-N // chunk) * chunk
    S = N_pad // N_CORES
    n_tiles = S // (P * R)
    VA = _to_u16(A, N_pad)
    VB = _to_u16(B, N_pad)

    def _m(V, i):
        # [S,2] u16 -> per-tile [lo block | hi block] layout, each P*R
        X = V[i * S : (i + 1) * S].reshape(n_tiles, P, R, 2)
        return np.ascontiguousarray(X.transpose(0, 3, 1, 2)).reshape(2 * S, 1)

    in_maps = [{"A": _m(VA, i), "B": _m(VB, i)} for i in range(N_CORES)]
    return in_maps, S


@functools.lru_cache(maxsize=None)
def _get_nc(S: int):
    return build_nc(S)


@functools.lru_cache(maxsize=None)
def bench_nc(S: int, reps: int):
    return build_nc(S, reps=reps, internal_out=True)


def kernel(A: np.ndarray, B: np.ndarray):
    N = np.asarray(A).shape[0]
    in_maps, S = prep_in_maps(A, B)
    nc = _get_nc(S)
    res = run_bass_kernel_spmd(nc, in_maps, list(range(N_CORES)))
    n_tiles = S // (P * R)
    gts, eqs = [], []
    for r in res.results:
        GE = r["GE"].reshape(n_tiles, 2, P * R)
        gts.append(GE[:, 0].reshape(S, 1))
        eqs.append(GE[:, 1].reshape(S, 1))
    gt = np.concatenate(gts, axis=0)[:N]
    eq = np.concatenate(eqs, axis=0)[:N]
    return gt.astype(np.float32), eq.astype(np.float32)
